# revision 1
# baseline (speedup 1.0000x reference)
"""GQA kernel for Trainium2, 8 NeuronCores, group-per-core sharding.

Reference: B=2, S=2048, D=2048, H=32 heads, G=8 kv groups (GS=4, HD=64).
Core g owns kv group g (4 heads). Host pre-transposes x and weight slices so
every device matmul contracts over the partition axis; host sums the 8
partial Wo projections.

Device layout (all "T" = transposed, contraction on partitions):
  QT[pair][b]  [128, 2048]  rows = 2 heads x 64 q-cols, cols = seq
  KT[b]        [64, 2048]   k^T ; VT[b] [64, 2048] v^T
  vaug[b][kt]  [128, 65]    v rows (natural) + ones col (softmax denominator)
  scores.T     [k=128, q=512] = KT_tile.T-matmul -> exp -> w (f32r)
  ctx.T psum   [65, 512]    = vaug.T @ w  (row 64 = softmax sums)
  out          [T=128, o=512] = ctxn_pair.T @ woT_pair  (natural layout)
Causal: only lower-triangular k-tiles computed; 4 diagonal tiles per
(head, q-chunk) get a 0/1 mask multiply.
"""
import numpy as np

import concourse.bacc as bacc
import concourse.mybir as mybir
import concourse.tile as tile
from concourse.bass_utils import run_bass_kernel_spmd

F32 = mybir.dt.float32
F32R = mybir.dt.float32r
AF = mybir.ActivationFunctionType

B, S, D = 2, 2048, 2048
G, GS, HD = 8, 4, 64
T = B * S            # 4096 flattened tokens
QCH = 512            # q-chunk (psum free dim)
NQC = S // QCH       # 4 q-chunks per batch
NKT = S // 128       # 16 k-tiles per batch
NTC = T // QCH       # 8 proj T-chunks
NKD = D // 128       # 16 contraction tiles over D


def build_nc():
    nc = bacc.Bacc("TRN2", target_bir_lowering=False, debug=False)
    xT = nc.dram_tensor("xT", [D, T], F32R, kind="ExternalInput")
    wqT = nc.dram_tensor("wqT", [D, GS * HD], F32R, kind="ExternalInput")
    wkvT = nc.dram_tensor("wkvT", [D, 2 * HD], F32R, kind="ExternalInput")
    woT = nc.dram_tensor("woT", [GS * HD, D], F32R, kind="ExternalInput")
    masks = nc.dram_tensor("masks", [128, 4 * QCH], F32R, kind="ExternalInput")
    aux = nc.dram_tensor("aux", [128, 128], F32R, kind="ExternalInput")
    outp = nc.dram_tensor("outp", [T, D], F32, kind="ExternalOutput")

    with tile.TileContext(nc) as tc:
        with tc.tile_pool(name="const", bufs=1) as const, \
             tc.tile_pool(name="store", bufs=1) as store:
            # --- static tiles -------------------------------------------------
            wq_sb = const.tile([128, NKD, GS * HD], F32R)
            nc.sync.dma_start(out=wq_sb[:], in_=xT_re(wqT, GS * HD))
            wkv_sb = const.tile([128, NKD, 2 * HD], F32R)
            nc.sync.dma_start(out=wkv_sb[:], in_=xT_re(wkvT, 2 * HD))
            wo_sb = [const.tile([128, D], F32R, tag=f"wo{p}", name=f"wo{p}") for p in range(2)]
            for p in range(2):
                nc.sync.dma_start(out=wo_sb[p][:], in_=woT[p * 128:(p + 1) * 128, :])
            mask_sb = const.tile([128, 4 * QCH], F32R)
            aux_sb = const.tile([128, 128], F32R)
            nc.sync.dma_start(out=aux_sb[:], in_=aux[:])
            nc.sync.dma_start(out=mask_sb[:], in_=masks[:])

            # long-lived activations
            QT = [[store.tile([64, 2 * S], F32R, tag=f"qt{p}{b}", name=f"qt{p}{b}") for b in range(B)]
                  for p in range(2)]
            KT = [store.tile([64, S], F32R, tag=f"kt{b}", name=f"ktt{b}") for b in range(B)]
            VT = [store.tile([64, S], F32R, tag=f"vt{b}", name=f"vtt{b}") for b in range(B)]
            vaug = [[store.tile([128, HD + 1], F32R, tag=f"va{b}_{kt}", name=f"va{b}_{kt}")
                     for kt in range(NKT)] for b in range(B)]

            # --- phase A: projections + v transpose --------------------------
            with tc.tile_pool(name="xp", bufs=18) as xp, \
                 tc.tile_pool(name="psA", bufs=2, space="PSUM") as psA:
                for tch in range(NTC):
                    b, col = tch // NQC, (tch % NQC) * QCH
                    xre = xT.rearrange("(kt p) t -> p kt t", p=128)
                    xt = []
                    for kt in range(NKD):
                        xk = xp.tile([128, QCH], F32R, tag="xt", name=f"xt{kt}")
                        nc.sync.dma_start(
                            out=xk[:],
                            in_=xre[:, kt, tch * QCH:(tch + 1) * QCH])
                        xt.append(xk)
                    for p in range(2):
                        ps_q = psA.tile([128, QCH], F32)
                        for kt in range(NKD):
                            nc.tensor.matmul(
                                ps_q[:], wq_sb[:, kt, p * 128:(p + 1) * 128],
                                xt[kt][:], start=(kt == 0), stop=(kt == NKD - 1))
                        nc.scalar.activation(
                            QT[p][b][:, col:col + QCH], ps_q[0:64, :], AF.Copy)
                        nc.scalar.activation(
                            QT[p][b][:, S + col:S + col + QCH], ps_q[64:128, :], AF.Copy)
                    ps_kv = psA.tile([128, QCH], F32)
                    for kt in range(NKD):
                        nc.tensor.matmul(ps_kv[:], wkv_sb[:, kt, :], xt[kt][:],
                                         start=(kt == 0), stop=(kt == NKD - 1))
                    nc.scalar.activation(KT[b][:, col:col + QCH], ps_kv[0:64, :], AF.Copy)
                    nc.scalar.activation(VT[b][:, col:col + QCH], ps_kv[64:128, :], AF.Copy)
                for b in range(B):
                    for kt in range(NKT):
                        ps_t = psA.tile([128, QCH], F32R, tag="pstr")
                        nc.tensor.transpose(
                            ps_t[:, 0:HD], VT[b][:, kt * 128:(kt + 1) * 128], aux_sb[0:64, 0:64])
                        nc.vector.tensor_copy(vaug[b][kt][:, 0:HD], ps_t[:, 0:HD])
                        nc.vector.tensor_copy(vaug[b][kt][:, HD:HD + 1], aux_sb[:, 64:65])

            # --- phase B: attention + output projection ----------------------
            with tc.tile_pool(name="wp", bufs=3) as wp, \
                 tc.tile_pool(name="cn", bufs=2) as cn, \
                 tc.tile_pool(name="rp", bufs=4) as rp, \
                 tc.tile_pool(name="ob", bufs=2) as ob, \
                 tc.tile_pool(name="pss", bufs=2, space="PSUM") as pss, \
                 tc.tile_pool(name="psc", bufs=2, space="PSUM") as psc, \
                 tc.tile_pool(name="psb", bufs=2, space="PSUM") as psb, \
                 tc.tile_pool(name="pso", bufs=2, space="PSUM") as pso:
                for b in range(B):
                    for qi in range(NQC):
                        kmax = 4 * (qi + 1)
                        ctxn = [cn.tile([128, QCH], F32R, tag=f"cn{p}", name=f"cn{p}") for p in range(2)]
                        for h in range(GS):
                            p, hb = h // 2, (h % 2) * 64
                            ps_ctx = psc.tile([HD + 1, QCH], F32)
                            pend = []  # software pipeline: delay MM2 by one k-tile
                            for kt in range(kmax):
                                ps_s = pss.tile([128, QCH], F32)
                                nc.tensor.matmul(
                                    ps_s[:], KT[b][:, kt * 128:(kt + 1) * 128],
                                    QT[p][b][:, (h % 2) * S + qi * QCH:(h % 2) * S + (qi + 1) * QCH],
                                    start=True, stop=True)
                                w = wp.tile([128, QCH], F32R)
                                nc.scalar.activation(w[:], ps_s[:], AF.Exp, scale=0.125)
                                dg = kt - 4 * qi
                                if dg >= 0:
                                    nc.vector.tensor_mul(
                                        w[:], w[:], mask_sb[:, dg * QCH:(dg + 1) * QCH])
                                pend.append((kt, w))
                                if len(pend) > 1:
                                    k0, w0 = pend.pop(0)
                                    nc.tensor.matmul(ps_ctx[:], vaug[b][k0][:], w0[:],
                                                     start=(k0 == 0), stop=False)
                            k0, w0 = pend.pop(0)
                            nc.tensor.matmul(ps_ctx[:], vaug[b][k0][:], w0[:],
                                             start=(k0 == 0), stop=True)
                            ctx_sb = rp.tile([64, QCH], F32, tag="cs")
                            nc.scalar.activation(ctx_sb[:], ps_ctx[0:64, :], AF.Copy)
                            rr = rp.tile([1, QCH], F32R, tag="rr")
                            with nc.allow_low_precision(reason="softmax recip f32r"):
                                nc.vector.reciprocal(rr[:], ps_ctx[64:65, :])
                            rbc_ps = psb.tile([64, QCH], F32)
                            nc.tensor.matmul(rbc_ps[:], aux_sb[0:1, 64:128], rr[:],
                                             start=True, stop=True)
                            nc.vector.tensor_mul(
                                ctxn[p][hb:hb + 64, :], ctx_sb[:], rbc_ps[:])
                        for tt in range(QCH // 128):
                            osb = ob.tile([128, D], F32)
                            for oc in range(D // 512):
                                ps_o = pso.tile([128, 512], F32)
                                for p in range(2):
                                    nc.tensor.matmul(
                                        ps_o[:], ctxn[p][:, tt * 128:(tt + 1) * 128],
                                        wo_sb[p][:, oc * 512:(oc + 1) * 512],
                                        start=(p == 0), stop=(p == 1))
                                nc.scalar.activation(
                                    osb[:, oc * 512:(oc + 1) * 512], ps_o[:], AF.Copy)
                            row = b * S + qi * QCH + tt * 128
                            nc.sync.dma_start(out=outp[row:row + 128, :], in_=osb[:])
    nc.compile()
    return nc


def xT_re(t, c):
    return t.rearrange("(kt p) c -> p kt c", p=128)


def prep_inputs(x, Wq, Wk, Wv, Wo):
    xT = np.ascontiguousarray(x.reshape(T, D).T)
    km = np.arange(128)[:, None]
    qm = np.arange(QCH)[None, :]
    masks = np.concatenate(
        [(128 * d + km <= qm).astype(np.float32) for d in range(4)], axis=1)
    aux = np.zeros((128, 128), dtype=np.float32)
    aux[:64, :64] = np.eye(64, dtype=np.float32)
    aux[:, 64:128] = 1.0
    in_maps = []
    for g in range(G):
        in_maps.append({
            "xT": xT,
            "wqT": np.ascontiguousarray(Wq[g * GS * HD:(g + 1) * GS * HD, :].T),
            "wkvT": np.ascontiguousarray(
                np.concatenate([Wk[g * HD:(g + 1) * HD, :],
                                Wv[g * HD:(g + 1) * HD, :]], axis=0).T),
            "woT": np.ascontiguousarray(Wo[:, g * GS * HD:(g + 1) * GS * HD].T),
            "masks": masks,
            "aux": aux,
        })
    return in_maps


def kernel(x, Wq, Wk, Wv, Wo):
    x = np.asarray(x, dtype=np.float32)
    in_maps = prep_inputs(np.asarray(x, np.float32), np.asarray(Wq, np.float32),
                          np.asarray(Wk, np.float32), np.asarray(Wv, np.float32),
                          np.asarray(Wo, np.float32))
    nc = build_nc()
    res = run_bass_kernel_spmd(nc, in_maps, list(range(G)))
    acc = np.zeros((T, D), dtype=np.float64)
    for g in range(G):
        acc += res.results[g]["outp"].astype(np.float64)
    return acc.astype(np.float32).reshape(B, S, D)



# revision 2
# speedup vs baseline: 1.2709x; 1.2709x over previous
"""GQA kernel for Trainium2, 8 NeuronCores, group-per-core sharding. v2.

Reference: B=2, S=2048, D=2048, H=32 heads, G=8 kv groups (GS=4, HD=64).
Core g owns kv group g (4 heads = 2 pairs). All SBUF data bf16; PSUM f32.

Layouts (per core):
  QT[p][b]   [128, S] bf16: rows 0-63 = head 2p Q^T (hd-major), 64-127 = head 2p+1
  K2T[b]     [128, S] bf16: rows 0-63 = K^T, rows 64-127 = same K^T (copy for
             the row-tiled second matmul of a pair)
  vaug[b][kt][128, 65] bf16: V natural rows + ones col (softmax denominator)
  scores     pair tile [128, 1024] f32 psum (2 banks): head A cols 0-512,
             head B 512-1024. MM1 = two concurrent row-tiled matmuls
             (contraction rows 0-63 / 64-127).
  exp        one ACT instr per pair tile -> w bf16 SBUF [128, 1024]
  MM2        per head: out = vaug.T @ w -> ctx psum [65, 512] (row 64 = denom)
  normalize  DVE recip + ones-matmul broadcast + DVE mul -> ctxn bf16
  out proj   ctxn.T @ woT -> psum [128,512], copy (ACT/DVE alternating) ->
             osb bf16 -> DMA out. Host sums the 8 cores' bf16 partials in f32.
Causal: only lower-triangular k-tiles; the 4 diagonal tiles per (pair,qi)
are q-restricted to [dg*128, 512) and get a [128,128] triangular mask mul.
"""
import numpy as np
import ml_dtypes

import concourse.bacc as bacc
import concourse.mybir as mybir
import concourse.tile as tile
from concourse.bass_utils import run_bass_kernel_spmd

F32 = mybir.dt.float32
F32R = mybir.dt.float32r
BF16 = mybir.dt.bfloat16
AF = mybir.ActivationFunctionType

B, S, D = 2, 2048, 2048
G, GS, HD = 8, 4, 64
T = B * S            # 4096 flattened tokens
QCH = 512            # q-chunk (psum free dim)
NQC = S // QCH       # 4 q-chunks per batch
NKT = S // 128       # 16 k-tiles per batch
NKD = D // 128       # 16 contraction tiles over D


def build_nc():
    nc = bacc.Bacc("TRN2", target_bir_lowering=False, debug=False)
    xT = nc.dram_tensor("xT", [D, T], BF16, kind="ExternalInput")
    wqT = nc.dram_tensor("wqT", [D, GS * HD], BF16, kind="ExternalInput")
    wkvT = nc.dram_tensor("wkvT", [D, 2 * HD], BF16, kind="ExternalInput")
    woT = nc.dram_tensor("woT", [GS * HD, D], BF16, kind="ExternalInput")
    masks = nc.dram_tensor("masks", [128, 256], BF16, kind="ExternalInput")
    auxb = nc.dram_tensor("auxb", [128, 128], BF16, kind="ExternalInput")
    auxr = nc.dram_tensor("auxr", [128, 64], F32R, kind="ExternalInput")
    outp = nc.dram_tensor("outp", [T, D], BF16, kind="ExternalOutput")

    with tile.TileContext(nc) as tc:
        with tc.tile_pool(name="const", bufs=1) as const, \
             tc.tile_pool(name="store", bufs=1) as store:
            # --- static tiles -------------------------------------------------
            wq_sb = const.tile([128, NKD, GS * HD], BF16)
            nc.sync.dma_start(out=wq_sb[:], in_=xT_re(wqT))
            wkv_sb = const.tile([128, NKD, 2 * HD], BF16)
            nc.sync.dma_start(out=wkv_sb[:], in_=xT_re(wkvT))
            wo_sb = [const.tile([128, D], BF16, tag=f"wo{p}", name=f"wo{p}")
                     for p in range(2)]
            for p in range(2):
                nc.sync.dma_start(out=wo_sb[p][:], in_=woT[p * 128:(p + 1) * 128, :])
            mask_sb = const.tile([128, 256], BF16)
            nc.sync.dma_start(out=mask_sb[:], in_=masks[:])
            auxb_sb = const.tile([128, 128], BF16)
            nc.sync.dma_start(out=auxb_sb[:], in_=auxb[:])
            auxr_sb = const.tile([128, 64], F32R)
            nc.sync.dma_start(out=auxr_sb[:], in_=auxr[:])

            # long-lived activations
            QT = [[store.tile([128, S], BF16, tag=f"qt{p}{b}", name=f"qt{p}{b}")
                   for b in range(B)] for p in range(2)]
            K2T = [store.tile([128, S], BF16, tag=f"kt{b}", name=f"ktt{b}")
                   for b in range(B)]
            VT = [store.tile([64, S], BF16, tag=f"vt{b}", name=f"vtt{b}")
                  for b in range(B)]
            vaug = [[store.tile([128, HD + 1], BF16, tag=f"va{b}_{kt}",
                                name=f"va{b}_{kt}")
                     for kt in range(NKT)] for b in range(B)]

            xre = xT.rearrange("(kt p) t -> p kt t", p=128)

            with tc.tile_pool(name="xp", bufs=18) as xp, \
                 tc.tile_pool(name="wp", bufs=3) as wp, \
                 tc.tile_pool(name="rp", bufs=2) as rp, \
                 tc.tile_pool(name="cn", bufs=2) as cn, \
                 tc.tile_pool(name="ob", bufs=2) as ob, \
                 tc.tile_pool(name="pss", bufs=2, space="PSUM") as pss, \
                 tc.tile_pool(name="psc", bufs=2, space="PSUM") as psc, \
                 tc.tile_pool(name="pst", bufs=2, space="PSUM") as pst:
                for b in range(B):
                    for qi in range(NQC):
                        # ---- projection chunk (b, qi): tokens col..col+512
                        col = qi * QCH
                        tch = b * NQC + qi
                        xt = []
                        for kt in range(NKD):
                            xk = xp.tile([128, QCH], BF16, tag="xt", name=f"xt{tch}_{kt}")
                            nc.sync.dma_start(
                                out=xk[:],
                                in_=xre[:, kt, tch * QCH:(tch + 1) * QCH])
                            xt.append(xk)
                        for p in range(2):
                            ps_q = pst.tile([128, QCH], F32, tag="tr")
                            for kt in range(NKD):
                                nc.tensor.matmul(
                                    ps_q[:], wq_sb[:, kt, p * 128:(p + 1) * 128],
                                    xt[kt][:], start=(kt == 0), stop=(kt == NKD - 1))
                            nc.vector.tensor_copy(QT[p][b][:, col:col + QCH], ps_q[:])
                        ps_kv = pst.tile([128, QCH], F32, tag="tr")
                        for kt in range(NKD):
                            nc.tensor.matmul(ps_kv[:], wkv_sb[:, kt, :], xt[kt][:],
                                             start=(kt == 0), stop=(kt == NKD - 1))
                        nc.vector.tensor_copy(K2T[b][0:64, col:col + QCH], ps_kv[0:64, :])
                        nc.vector.tensor_copy(K2T[b][64:128, col:col + QCH], ps_kv[0:64, :])
                        nc.vector.tensor_copy(VT[b][:, col:col + QCH], ps_kv[64:128, :])
                        # v transposes for the 4 new k-tiles
                        for j in range(4):
                            kt = qi * 4 + j
                            ps_t = pst.tile([128, HD], BF16, tag="tr")
                            nc.tensor.transpose(
                                ps_t[:], VT[b][:, kt * 128:(kt + 1) * 128],
                                auxb_sb[0:64, 0:64])
                            nc.vector.tensor_copy(vaug[b][kt][:, 0:HD], ps_t[:])
                            nc.vector.tensor_copy(vaug[b][kt][:, HD:HD + 1],
                                                  auxb_sb[:, 64:65])

                        # ---- attention for (b, qi), pair-sequential ----------
                        kmax = 4 * (qi + 1)
                        ctxn = [cn.tile([128, QCH], BF16, tag=f"cn{p}", name=f"cn{tch}_{p}")
                                for p in range(2)]
                        for p in range(2):
                            ctx = [psc.tile([HD + 1, QCH], F32, tag="ctx",
                                            name=f"ctx{tch}_{p}_{h}")
                                   for h in range(2)]
                            pend = []
                            for kt in range(kmax):
                                dg = kt - 4 * qi
                                q0 = dg * 128 if dg >= 0 else 0
                                qw = QCH - q0
                                ps_s = pss.tile([128, 2 * QCH], F32, tag="sc")
                                for h in range(2):
                                    r0 = h * 64
                                    nc.tensor.matmul(
                                        ps_s[:, h * QCH + q0:(h + 1) * QCH],
                                        K2T[b][r0:r0 + 64, kt * 128:(kt + 1) * 128],
                                        QT[p][b][r0:r0 + 64, col + q0:col + QCH],
                                        start=True, stop=True)
                                w = wp.tile([128, 2 * QCH], BF16, tag="w")
                                if q0:
                                    ps_v = ps_s[:].rearrange(
                                        "p (h q) -> p h q", h=2)[:, :, q0:QCH]
                                    w_v = w[:].rearrange(
                                        "p (h q) -> p h q", h=2)[:, :, q0:QCH]
                                    nc.scalar.activation(w_v, ps_v, AF.Exp, scale=0.125)
                                else:
                                    nc.scalar.activation(w[:], ps_s[:], AF.Exp,
                                                         scale=0.125)
                                if dg >= 0:
                                    wm = w[:].rearrange(
                                        "p (h q) -> p h q", h=2)[:, :, q0:q0 + 128]
                                    mk = mask_sb[:].rearrange(
                                        "p (h q) -> p h q", h=2)
                                    nc.vector.tensor_mul(wm, wm, mk)
                                pend.append((kt, q0, w))
                                if len(pend) > 1:
                                    mm2(nc, pend.pop(0), ctx, vaug[b], qi)
                            mm2(nc, pend.pop(0), ctx, vaug[b], qi)
                            # epilogue: normalize both heads of the pair
                            rr = rp.tile([128, QCH], F32R, tag="rr")
                            for h in range(2):
                                row = 64 + 32 * h
                                with nc.allow_low_precision(reason="softmax recip"):
                                    nc.vector.reciprocal(
                                        rr[row:row + 1, :], ctx[h][64:65, :])
                                rbc = pst.tile([64, QCH], F32, tag="tr")
                                nc.tensor.matmul(
                                    rbc[:], auxr_sb[row:row + 1, :],
                                    rr[row:row + 1, :], start=True, stop=True,
                                    tile_position=(row, 0))
                                cs = rp.tile([64, QCH], F32, tag="cs")
                                nc.scalar.activation(cs[:], ctx[h][0:64, :], AF.Copy)
                                nc.vector.tensor_mul(
                                    ctxn[p][h * 64:(h + 1) * 64, :],
                                    cs[:], rbc[:])
                        # ---- output projection for this 512-token chunk ------
                        for tt in range(QCH // 128):
                            osb = ob.tile([128, D], BF16, tag="osb")
                            for oc in range(D // 512):
                                ps_o = pst.tile([128, 512], F32, tag="tr")
                                for p in range(2):
                                    nc.tensor.matmul(
                                        ps_o[:], ctxn[p][:, tt * 128:(tt + 1) * 128],
                                        wo_sb[p][:, oc * 512:(oc + 1) * 512],
                                        start=(p == 0), stop=(p == 1))
                                dst = osb[:, oc * 512:(oc + 1) * 512]
                                if (tt + oc) % 2 == 0:
                                    nc.scalar.activation(dst, ps_o[:], AF.Copy)
                                else:
                                    nc.vector.tensor_copy(dst, ps_o[:])
                            row = b * S + qi * QCH + tt * 128
                            nc.sync.dma_start(out=outp[row:row + 128, :], in_=osb[:])
    nc.compile()
    return nc


def mm2(nc, item, ctx, vaug_b, qi):
    kt, q0, w = item
    for h in range(2):
        nc.tensor.matmul(
            ctx[h][:, q0:], vaug_b[kt][:],
            w[:, h * QCH + q0:(h + 1) * QCH],
            start=(kt == 0), stop=(kt == 4 * (qi + 1) - 1))


def xT_re(t):
    return t.rearrange("(kt p) c -> p kt c", p=128)


def prep_inputs(x, Wq, Wk, Wv, Wo):
    bf = ml_dtypes.bfloat16
    xT = np.ascontiguousarray(x.reshape(T, D).T).astype(bf)
    km = np.arange(128)[:, None]
    qm = np.arange(128)[None, :]
    tri = (km <= qm).astype(np.float32)
    masks = np.concatenate([tri, tri], axis=1).astype(bf)
    auxb = np.zeros((128, 128), dtype=np.float32)
    auxb[:64, :64] = np.eye(64, dtype=np.float32)
    auxb[:, 64:128] = 1.0
    auxr = np.ones((128, 64), dtype=np.float32)
    in_maps = []
    for g in range(G):
        in_maps.append({
            "xT": xT,
            "wqT": np.ascontiguousarray(Wq[g * GS * HD:(g + 1) * GS * HD, :].T).astype(bf),
            "wkvT": np.ascontiguousarray(
                np.concatenate([Wk[g * HD:(g + 1) * HD, :],
                                Wv[g * HD:(g + 1) * HD, :]], axis=0).T).astype(bf),
            "woT": np.ascontiguousarray(Wo[:, g * GS * HD:(g + 1) * GS * HD].T).astype(bf),
            "masks": masks,
            "auxb": auxb.astype(bf),
            "auxr": auxr,
        })
    return in_maps


def kernel(x, Wq, Wk, Wv, Wo):
    in_maps = prep_inputs(np.asarray(x, np.float32), np.asarray(Wq, np.float32),
                          np.asarray(Wk, np.float32), np.asarray(Wv, np.float32),
                          np.asarray(Wo, np.float32))
    nc = build_nc()
    res = run_bass_kernel_spmd(nc, in_maps, list(range(G)))
    acc = np.zeros((T, D), dtype=np.float32)
    for g in range(G):
        acc += res.results[g]["outp"].astype(np.float32)
    return acc.reshape(B, S, D)
